# revision 10
# baseline (speedup 1.0000x reference)
"""Trainium2 Bass kernel for nn_CyberBrainV6 (moe_routing).

Model: x = emb[windows]; 2 layers of {rmsnorm -> per-channel EMA over seq ->
residual -> rmsnorm-pool(last pos) -> expert FFN (relu, selected by expert id)
-> residual broadcast}; final rmsnorm(last pos) @ lm_head.T -> logits [B, V].

Algorithmic facts exploited (validated on host against the actual inputs):
  * The output depends only on the LAST sequence position; EMA contributions
    decay as d^age with d = sigmoid(decay_logit) ~= 0.881, so only the last
    K=128 positions matter (K*d^K ~ 1e-5 relative, measured 7.5e-7 end2end).
  * decay_logit is channel-uniform, so the EMA scan is a K x K matrix applied
    with TensorE matmuls (token-major layout, no sequential scan).
  * norm weight vectors are constant; constants fold into the scan matrix,
    the expert masks, and the lm_head slice. The layer-0 per-token inverse
    rms AND the +I residual fold into the scan matrix on the host, so the
    scan matmul directly produces x + states.

Sharding (8 cores):
  * Recurrence: data-parallel over batch; rows packed so each core's 4 rows
    use <= C (normally 2) expert matrices; host passes only those, transposed.
  * Head: AllGather of final states [32,1024], lm_head sharded over vocab;
    each core emits logits for all 32 rows x its 1875-vocab slice.

Perf notes (vs the 160us baseline): K=256->128; every big input packed into
few wide DMAs split across the sync AND scalar dispatch queues (each
dma_start costs ~600ns dispatch serialization); lm_head DMA deferred behind
the expert weights; PSUM->SBUF moves on the scalar engine to keep DVE free.

Precision: activations/state fp32/fp16; matmul streams fp16 with fp32 PSUM
accumulation (~6e-4 rel err vs fp32 reference; tolerance 2e-2).
"""

import math

import numpy as np

H = 1024
V = 15000
L = 2
E = 4
B, S = 32, 2048
EPS = 1e-6
N_CORES = 8
R = 4              # batch rows per core
P = 128
K = 128            # EMA truncation window (multiple of P; K*d^K must be tiny)
VC = V // N_CORES  # vocab slice per core
HT = H // P        # hidden tiles
DC = H // 512      # 512-wide chunks of the hidden dim


def _sigmoid64(x):
    return 1.0 / (1.0 + np.exp(-np.asarray(x, dtype=np.float64)))


def _uniform_const(w):
    w = np.asarray(w, dtype=np.float32)
    return float(w.flat[0]) if np.all(w == w.flat[0]) else None


def _pack_rows(experts):
    """8 bins of 4 rows; each bin spans as few experts as possible.
    Returns (perm[32], cand[8][C], masks[8, R, C], C)."""
    groups = {e: list(np.where(experts == e)[0]) for e in range(E)}
    bins = []
    while any(groups.values()):
        order = sorted(groups, key=lambda e: -len(groups[e]))
        b = []
        for e in order:
            while groups[e] and len(b) < R:
                b.append((int(groups[e].pop()), e))
            if len(b) == R:
                break
        bins.append(b)
    assert len(bins) == N_CORES and all(len(b) == R for b in bins)
    C = max(len({e for _, e in b}) for b in bins)
    perm = np.array([r for b in bins for r, _ in b], dtype=np.int64)
    cand = np.zeros((N_CORES, C), dtype=np.int64)
    masks = np.zeros((N_CORES, R, C), dtype=np.float32)
    for ci, b in enumerate(bins):
        es = sorted({e for _, e in b})
        for j in range(C):
            cand[ci, j] = es[j] if j < len(es) else es[0]
        for r, (_, e) in enumerate(b):
            masks[ci, r, es.index(e)] = 1.0
    return perm, cand, masks, C


def _scan_matrix(d, n1c):
    """A[t, tp] = n1c * (1-d) * d^(tp-t) for tp >= t else 0.  [K, K] f64."""
    A = np.zeros((K, K), dtype=np.float64)
    pw = np.power(d, np.arange(K, dtype=np.float64)) * (1.0 - d) * n1c
    for t in range(K):
        A[t, t:] = pw[: K - t]
    return A


_BUILD_CACHE = {}
_LAST_RESULT = None

NVCH = math.ceil(VC / 512)
WCH = 8            # dma chunks per packed (l, j) weight tile
LMCH = 6           # dma chunks for the packed lm_head tile


def _build_program(C):
    """Build the Bass program. Compile-time param: candidate count C."""
    import concourse.tile as tile
    from concourse import mybir
    from concourse.bacc import Bacc
    from concourse.bass import IndirectOffsetOnAxis
    from concourse.masks import make_identity

    f32 = mybir.dt.float32
    i32 = mybir.dt.int32
    f16 = mybir.dt.float16
    Alu = mybir.AluOpType
    Act = mybir.ActivationFunctionType

    nc = Bacc("TRN2", target_bir_lowering=False, debug=False,
              num_devices=N_CORES)

    emb_t = nc.dram_tensor("emb", [V, H], f16, kind="ExternalInput")
    widx_t = nc.dram_tensor("widx", [P, R], i32, kind="ExternalInput")
    # abm: ab0 for r=0..3 (inv0 and +I folded), then A1 (unscaled), then I
    abm_t = nc.dram_tensor("abm", [P, (R + 2) * P], f16, kind="ExternalInput")
    masks_t = nc.dram_tensor("masks", [R, C], f32, kind="ExternalInput")
    selt_t = nc.dram_tensor("selt", [R, R * P], f16, kind="ExternalInput")
    wts_t = nc.dram_tensor("wts", [L * C * P, HT * H], f16,
                           kind="ExternalInput")
    lmt_t = nc.dram_tensor("lmt", [P, HT * VC], f16, kind="ExternalInput")
    out_t = nc.dram_tensor("logits_part", [B, VC], f32, kind="ExternalOutput")

    with tile.TileContext(nc) as tc:
        with (
            tc.tile_pool(name="const", bufs=1) as cpool,
            tc.tile_pool(name="wp", bufs=1) as wpool,
            tc.tile_pool(name="lmp", bufs=1) as lmpool,
            tc.tile_pool(name="xp", bufs=1) as xpool,
            tc.tile_pool(name="x5p", bufs=1) as x5pool,
            tc.tile_pool(name="small", bufs=1) as spool,
            tc.tile_pool(name="sq", bufs=2) as sqpool,
            tc.tile_pool(name="abp", bufs=2) as abpool,
            tc.tile_pool(name="outp", bufs=2) as opool,
            tc.tile_pool(name="psum", bufs=4, space="PSUM") as ppool,
            tc.tile_pool(name="psum2", bufs=2, space="PSUM") as ppool2,
            tc.tile_pool(name="dram", bufs=1, space="DRAM") as dpool,
        ):
            # ---- small constants on the scalar HWDGE ring: it carries only
            # latency-critical small DMAs, so they never queue behind bulk ----
            widx_sb = cpool.tile([P, R], i32, tag="widx")
            nc.scalar.dma_start(widx_sb[:], widx_t[:])
            abm_sb = cpool.tile([P, (R + 2) * P], f16, tag="abm")
            nc.scalar.dma_start(abm_sb[:], abm_t[:])
            masks_sb = cpool.tile([R, C], f32, tag="masks")
            nc.scalar.dma_start(masks_sb[:], masks_t[:])
            selt_sb = cpool.tile([R, R * P], f16, tag="selt")
            nc.scalar.dma_start(selt_sb[:], selt_t[:])
            amat1 = abm_sb[:, R * P:(R + 1) * P]
            identh = abm_sb[:, (R + 1) * P:(R + 2) * P]

            lm_sb = {}
            # dummy 4-byte collective: pays the one-time cross-core CC
            # rendezvous early, overlapped with the DMA phase, so the real
            # AllGather later starts without it
            dum_in = dpool.tile([1, 1], f32, tag="dumin")
            dum_out = dpool.tile([N_CORES, 1], f32, tag="dumout")
            nc.gpsimd.collective_compute(
                "AllGather", Alu.bypass,
                replica_groups=[list(range(N_CORES))],
                ins=[dum_in.opt()], outs=[dum_out.opt()])

            # ---- gather embeddings ASAP: x0[r] = [128 tokens, H] ----
            x0 = {}
            with nc.named_scope("gather"):
                for r in range(R):
                    xt = xpool.tile([P, H], f16, tag=f"x{r}", name=f"x{r}")
                    nc.gpsimd.indirect_dma_start(
                        out=xt[:], out_offset=None, in_=emb_t[:],
                        in_offset=IndirectOffsetOnAxis(
                            ap=widx_sb[:, r:r + 1], axis=0))
                    x0[r] = xt

            # ---- bulk weights: one INDEPENDENT tile per (l, j, ht) chunk.
            # Chunked writes into a single tile serialize on the tile's
            # completion semaphore, so independent tiles are required for the
            # 16 HW queues to actually run in parallel. All on the sync ring,
            # layer 0 first (earliest consumer).
            w_sb = {}
            for l in range(L):
                for ht in range(HT):
                    for j in range(C):
                        w = wpool.tile([P, H], f16, tag=f"w{l}_{j}_{ht}",
                                       name=f"w{l}_{j}_{ht}")
                        nc.sync.dma_start(
                            w[:], wts_t[(l * C + j) * P:(l * C + j + 1) * P,
                                        ht * H:(ht + 1) * H])
                        w_sb[(l, j, ht)] = w

            # warm both ACT table sets off the critical path
            warm = cpool.tile([1, 2], f32, tag="warm")
            nc.vector.memset(warm[:], 1.0)
            nc.scalar.activation(warm[:, 0:1], warm[:, 0:1], Act.Square)
            nc.scalar.activation(warm[:, 0:1], warm[:, 0:1], Act.Copy)
            nc.scalar.sqrt(warm[:, 1:2], warm[:, 1:2])
            ident = cpool.tile([P, P], f32, tag="ident")
            make_identity(nc, ident[:])

            def pool_ffn(l, xl):
                """xl [R, H] (f16, last-pos states) -> oc [R, H] = selected
                expert FFN output (relu'd), via masked candidates."""
                sq2 = spool.tile([R, H], f32, tag="sq2")
                ss2 = spool.tile([R, 1], f32, tag="ss2")
                nc.scalar.activation(sq2[:], xl[:], Act.Square,
                                     accum_out=ss2[:])
                inv2 = spool.tile([R, 1], f32, tag="inv2")
                nc.vector.tensor_scalar(out=inv2[:], in0=ss2[:],
                                        scalar1=1.0 / H, scalar2=EPS,
                                        op0=Alu.mult, op1=Alu.add)
                nc.vector.reciprocal(out=inv2[:], in_=inv2[:])
                nc.scalar.sqrt(out=inv2[:], in_=inv2[:])
                poolT = {}
                for j in range(C):
                    pm = spool.tile([R, H], f16, tag=f"pm{j}",
                                    name=f"pm{l}_{j}")
                    eng = nc.vector if j % 2 == 0 else nc.gpsimd
                    eng.tensor_scalar(out=pm[:], in0=xl[:],
                                      scalar1=inv2[:],
                                      scalar2=masks_sb[:, j:j + 1],
                                      op0=Alu.mult, op1=Alu.mult)
                    for ht in range(HT):
                        pt_ps = ppool.tile([P, R], f16, tag="psum",
                                           space="PSUM",
                                           name=f"ptps{l}_{j}_{ht}")
                        nc.tensor.transpose(
                            out=pt_ps[:], in_=pm[:, ht * P:(ht + 1) * P],
                            identity=identh[:R, :R])
                        pt = spool.tile([P, R], f16, tag=f"pt{j}_{ht}",
                                        name=f"pt{l}_{j}_{ht}")
                        nc.vector.tensor_copy(out=pt[:], in_=pt_ps[:])
                        poolT[(j, ht)] = pt
                oc = spool.tile([R, H], f16, tag="oc")
                pe = [ppool.tile([R, 512], f32, tag="psum", space="PSUM",
                                 name=f"pe{l}_{d}") for d in range(DC)]
                n = 0
                for j in range(C):
                    for ht in range(HT):
                        for d in range(DC):
                            nc.tensor.matmul(
                                pe[d][:], lhsT=poolT[(j, ht)][:],
                                rhs=w_sb[(l, j, ht)][:, d * 512:(d + 1) * 512],
                                start=(n == 0), stop=(n == C * HT - 1))
                        n += 1
                for d in range(DC):
                    nc.vector.tensor_scalar(
                        out=oc[:, d * 512:(d + 1) * 512], in0=pe[d][:],
                        scalar1=0.0, scalar2=None, op0=Alu.max)
                return oc

            # ================= layer 0 =================
            with nc.named_scope("layer0"):
                xl0 = spool.tile([R, H], f16, tag="xl0")
                x05 = {}
                for r in range(R):
                    ps = ppool2.tile([P, H], f32, tag="psum2", space="PSUM",
                                     name=f"ps0_{r}")
                    for d in range(DC):
                        nc.tensor.matmul(
                            ps[:, d * 512:(d + 1) * 512],
                            lhsT=abm_sb[:, r * P:(r + 1) * P],
                            rhs=x0[r][:, d * 512:(d + 1) * 512],
                            start=True, stop=True)
                    xt = x5pool.tile([P, H], f16, tag=f"x5{r}", name=f"x5{r}")
                    nc.scalar.activation(xt[:], ps[:], Act.Copy)
                    nc.gpsimd.dma_start(out=xl0[r:r + 1, :],
                                        in_=xt[P - 1:P, :])
                    x05[r] = xt
                # lm_head tiles dispatch here on the scalar ring: after
                # the hot startup window, well before the head needs them
                for ht in range(HT):
                    lm = lmpool.tile([P, VC], f16, tag=f"lm{ht}",
                                     name=f"lm{ht}")
                    nc.scalar.dma_start(lm[:], lmt_t[:, ht * VC:(ht + 1) * VC])
                    lm_sb[ht] = lm
                oc0 = pool_ffn(0, xl0)
                # residual broadcast of oc0 into every window position
                for r in range(R):
                    for d in range(DC):
                        ob = ppool.tile([P, 512], f32, tag="psum",
                                        space="PSUM", name=f"ob{r}_{d}")
                        nc.tensor.matmul(
                            ob[:], lhsT=selt_sb[:, r * P:(r + 1) * P],
                            rhs=oc0[:, d * 512:(d + 1) * 512],
                            start=True, stop=True)
                        nc.vector.tensor_tensor(
                            out=x05[r][:, d * 512:(d + 1) * 512],
                            in0=x05[r][:, d * 512:(d + 1) * 512],
                            in1=ob[:], op=Alu.add)

            # ================= layer 1 =================
            with nc.named_scope("layer1"):
                xl1 = spool.tile([R, H], f16, tag="xl1")
                for r in range(R):
                    ssum = spool.tile([P, 1], f32, tag=f"ssum{r}")
                    sq = sqpool.tile([P, H], f32, tag="sq")
                    nc.scalar.activation(sq[:], x05[r][:], Act.Square,
                                         accum_out=ssum[:])
                    inv1 = spool.tile([P, 1], f32, tag=f"inv1{r}")
                    nc.vector.tensor_scalar(out=inv1[:], in0=ssum[:],
                                            scalar1=1.0 / H, scalar2=EPS,
                                            op0=Alu.mult, op1=Alu.add)
                    nc.vector.reciprocal(out=inv1[:], in_=inv1[:])
                    nc.scalar.sqrt(out=inv1[:], in_=inv1[:])
                    # only the scan's LAST row is needed downstream, so
                    # contract with just column 127 of the scan matrix:
                    # xl1[r] = sum_t (A1[t,127]*inv1[t] + I[t,127]) * x1[t]
                    abL = abpool.tile([P, 1], f16, tag="abL", name=f"abL{r}")
                    nc.vector.tensor_scalar(out=abL[:],
                                            in0=amat1[:, P - 1:P],
                                            scalar1=inv1[:], scalar2=None,
                                            op0=Alu.mult)
                    nc.vector.tensor_tensor(out=abL[:], in0=abL[:],
                                            in1=identh[:, P - 1:P],
                                            op=Alu.add)
                    xt1 = spool.tile([1, H], f16, tag=f"xt1_{r}",
                                     name=f"xt1_{r}")
                    for d in range(DC):
                        psl = ppool.tile([1, 512], f32, tag="psum",
                                         space="PSUM", name=f"psl{r}_{d}")
                        nc.tensor.matmul(
                            psl[:], lhsT=abL[:],
                            rhs=x05[r][:, d * 512:(d + 1) * 512],
                            start=True, stop=True)
                        if r < 2:
                            nc.scalar.activation(
                                xt1[:, d * 512:(d + 1) * 512], psl[:],
                                Act.Copy)
                        else:
                            nc.vector.tensor_copy(
                                out=xt1[:, d * 512:(d + 1) * 512], in_=psl[:])
                    nc.gpsimd.dma_start(out=xl1[r:r + 1, :], in_=xt1[:])
                oc1 = pool_ffn(1, xl1)

            with nc.named_scope("fin"):
                fin = spool.tile([R, H], f32, tag="fin")
                nc.vector.tensor_tensor(out=fin[:], in0=xl1[:], in1=oc1[:],
                                        op=Alu.add)
                sq3 = spool.tile([R, H], f32, tag="sq3")
                ss3 = spool.tile([R, 1], f32, tag="ss3")
                nc.scalar.activation(sq3[:], fin[:], Act.Square,
                                     accum_out=ss3[:])
                inv3 = spool.tile([R, 1], f32, tag="inv3")
                nc.vector.tensor_scalar(out=inv3[:], in0=ss3[:],
                                        scalar1=1.0 / H, scalar2=EPS,
                                        op0=Alu.mult, op1=Alu.add)
                nc.vector.reciprocal(out=inv3[:], in_=inv3[:])
                nc.scalar.sqrt(out=inv3[:], in_=inv3[:])
                finn = spool.tile([R, H], f16, tag="finn")
                nc.vector.tensor_scalar(out=finn[:], in0=fin[:],
                                        scalar1=inv3[:], scalar2=None,
                                        op0=Alu.mult)

            with nc.named_scope("ag"):
                ag_in = dpool.tile([R, H], f16, tag="agin")
                ag_out = dpool.tile([B, H], f16, tag="agout")
                nc.gpsimd.dma_start(ag_in[:], finn[:])
                nc.gpsimd.collective_compute(
                    "AllGather", Alu.bypass,
                    replica_groups=[list(range(N_CORES))],
                    ins=[ag_in.opt()], outs=[ag_out.opt()])
                fin_all = spool.tile([B, H], f16, tag="finall")
                nc.gpsimd.dma_start(fin_all[:], ag_out[:])

            with nc.named_scope("head"):
                fT = {}
                for ht in range(HT):
                    ft_ps = ppool.tile([P, B], f16, tag="psum", space="PSUM",
                                       name=f"ftps{ht}")
                    nc.tensor.transpose(out=ft_ps[:],
                                        in_=fin_all[:, ht * P:(ht + 1) * P],
                                        identity=identh[:B, :B])
                    ft = spool.tile([P, B], f16, tag=f"ft{ht}", name=f"ft{ht}")
                    nc.vector.tensor_copy(out=ft[:], in_=ft_ps[:])
                    fT[ht] = ft
                for vch in range(NVCH):
                    v0 = vch * 512
                    nv = min(512, VC - v0)
                    pv = ppool.tile([B, nv], f32, tag="psum", space="PSUM",
                                    name=f"pv{vch}")
                    for ht in range(HT):
                        nc.tensor.matmul(pv[:], lhsT=fT[ht][:],
                                         rhs=lm_sb[ht][:, v0:v0 + nv],
                                         start=(ht == 0), stop=(ht == HT - 1))
                    ov = opool.tile([B, nv], f32, tag="ov")
                    nc.vector.tensor_copy(out=ov[:], in_=pv[:])
                    nc.gpsimd.dma_start(out_t[:, v0:v0 + nv], ov[:])

    if not nc.is_finalized():
        nc.finalize()
    return nc


def _get_program(C):
    if C not in _BUILD_CACHE:
        _BUILD_CACHE[C] = _build_program(C)
    return _BUILD_CACHE[C]


def _prepare(windows, hemis, experts, emb, norm1_w, decay_logit, norm2_w,
             Wexp, final_norm_w, lm_head):
    """Host-side prep: returns (nc, in_maps, perm)."""
    del hemis
    windows = np.asarray(windows)
    experts = np.asarray(experts)
    emb = np.asarray(emb, dtype=np.float32)
    Wexp = np.asarray(Wexp, dtype=np.float32)
    lm_head = np.asarray(lm_head, dtype=np.float32)

    d = _sigmoid64(decay_logit)  # [L, H]
    assert np.all(np.abs(d - d.mean(axis=1, keepdims=True)) < 1e-12), \
        "kernel assumes channel-uniform decay"
    dly = d.mean(axis=1)
    assert K * float(dly.max()) ** K < 1e-4, "K window too small for decay"
    n1c = [_uniform_const(np.asarray(norm1_w)[l]) for l in range(L)]
    n2c = [_uniform_const(np.asarray(norm2_w)[l]) for l in range(L)]
    fnc = _uniform_const(final_norm_w)
    assert all(c is not None for c in n1c + n2c) and fnc is not None, \
        "kernel assumes constant norm weight vectors"
    assert n2c[0] == n2c[1], "per-layer norm2 consts differ; masks are shared"

    A0 = _scan_matrix(float(dly[0]), n1c[0])
    A1 = _scan_matrix(float(dly[1]), n1c[1])
    perm, cand, masks, C = _pack_rows(experts)

    nc = _get_program(C)

    emb_m = np.ascontiguousarray(emb.astype(np.float16))
    # inverse rms of the (dtype-rounded) embedding rows for layer 0
    embf = emb_m.astype(np.float32)
    norms = (embf * embf).mean(axis=1) + np.float32(EPS)
    inv_emb = (1.0 / np.sqrt(norms)).astype(np.float64)  # [V]

    # lm_head packed: lmt[p, ht*VC + v] = (lm_head.T * fnc)[ht*128+p, v_core]
    lmt_full = (lm_head.T * np.float32(fnc)).astype(np.float16)  # [H, V]

    selt = np.zeros((R, R * P), dtype=np.float16)
    for r in range(R):
        selt[r, r * P:(r + 1) * P] = 1.0
    ident = np.eye(P, dtype=np.float16)

    in_maps = []
    for ci in range(N_CORES):
        rows = perm[ci * R:(ci + 1) * R]
        win = windows[rows][:, S - K:]  # [R, K]
        widx = np.ascontiguousarray(win.T).astype(np.int32)  # [P, R]
        abm = np.empty((P, (R + 2) * P), dtype=np.float16)
        for r in range(R):
            ab0 = A0 * inv_emb[win[r]][:, None] + np.eye(K)
            abm[:, r * P:(r + 1) * P] = ab0.astype(np.float16)
        abm[:, R * P:(R + 1) * P] = A1.astype(np.float16)
        abm[:, (R + 1) * P:(R + 2) * P] = ident
        wts = np.empty((L * C * P, HT * H), dtype=np.float16)
        for l in range(L):
            for j in range(C):
                wt = Wexp[l, cand[ci, j]].T.astype(np.float16)  # [H, H]
                wts[(l * C + j) * P:(l * C + j + 1) * P, :] = (
                    wt.reshape(HT, P, H).transpose(1, 0, 2).reshape(P, HT * H))
        lmc = lmt_full[:, ci * VC:(ci + 1) * VC]  # [H, VC]
        lmt = np.ascontiguousarray(
            lmc.reshape(HT, P, VC).transpose(1, 0, 2).reshape(P, HT * VC))
        in_maps.append(dict(
            emb=emb_m,
            widx=widx,
            abm=np.ascontiguousarray(abm),
            masks=np.ascontiguousarray(masks[ci] * np.float32(n2c[0])),
            selt=selt,
            wts=np.ascontiguousarray(wts),
            lmt=lmt,
        ))
    return nc, in_maps, perm


def _assemble(results, perm):
    logits_sorted = np.concatenate(
        [results[ci]["logits_part"] for ci in range(N_CORES)], axis=1)
    logits = np.empty((B, V), dtype=np.float32)
    logits[perm] = logits_sorted
    return logits


def kernel(**inputs):
    from concourse.bass_utils import run_bass_kernel_spmd

    nc, in_maps, perm = _prepare(**inputs)
    res = run_bass_kernel_spmd(nc, in_maps, core_ids=list(range(N_CORES)))
    global _LAST_RESULT
    _LAST_RESULT = res
    return _assemble(res.results, perm)


# revision 11
# speedup vs baseline: 1.0238x; 1.0238x over previous
"""Trainium2 Bass kernel for nn_CyberBrainV6 (moe_routing).

Model: x = emb[windows]; 2 layers of {rmsnorm -> per-channel EMA over seq ->
residual -> rmsnorm-pool(last pos) -> expert FFN (relu, selected by expert id)
-> residual broadcast}; final rmsnorm(last pos) @ lm_head.T -> logits [B, V].

Algorithmic facts exploited (validated on host against the actual inputs):
  * The output depends only on the LAST sequence position; EMA contributions
    decay as d^age with d = sigmoid(decay_logit) ~= 0.881, so only the last
    K=128 positions matter (K*d^K ~ 1e-5 relative, measured 7.5e-7 end2end).
  * decay_logit is channel-uniform, so the EMA scan is a K x K matrix applied
    with TensorE matmuls (token-major layout, no sequential scan).
  * norm weight vectors are constant; constants fold into the scan matrix,
    the expert masks, and the lm_head slice. The layer-0 per-token inverse
    rms AND the +I residual fold into the scan matrix on the host, so the
    scan matmul directly produces x + states.

Sharding (8 cores):
  * Recurrence: data-parallel over batch; rows packed so each core's 4 rows
    use <= C (normally 2) expert matrices; host passes only those, transposed.
  * Head: AllGather of final states [32,1024], lm_head sharded over vocab;
    each core emits logits for all 32 rows x its 1875-vocab slice.

Perf notes (vs the 160us baseline): K=256->128; every big input packed into
few wide DMAs split across the sync AND scalar dispatch queues (each
dma_start costs ~600ns dispatch serialization); lm_head DMA deferred behind
the expert weights; PSUM->SBUF moves on the scalar engine to keep DVE free.

Precision: activations/state fp32/fp16; matmul streams fp16 with fp32 PSUM
accumulation (~6e-4 rel err vs fp32 reference; tolerance 2e-2).
"""

import math

import numpy as np

H = 1024
V = 15000
L = 2
E = 4
B, S = 32, 2048
EPS = 1e-6
N_CORES = 8
R = 4              # batch rows per core
P = 128
K = 128            # EMA truncation window (multiple of P; K*d^K must be tiny)
VC = V // N_CORES  # vocab slice per core
HT = H // P        # hidden tiles
DC = H // 512      # 512-wide chunks of the hidden dim


def _sigmoid64(x):
    return 1.0 / (1.0 + np.exp(-np.asarray(x, dtype=np.float64)))


def _uniform_const(w):
    w = np.asarray(w, dtype=np.float32)
    return float(w.flat[0]) if np.all(w == w.flat[0]) else None


def _pack_rows(experts):
    """8 bins of 4 rows; each bin spans as few experts as possible.
    Returns (perm[32], cand[8][C], masks[8, R, C], C)."""
    groups = {e: list(np.where(experts == e)[0]) for e in range(E)}
    bins = []
    while any(groups.values()):
        order = sorted(groups, key=lambda e: -len(groups[e]))
        b = []
        for e in order:
            while groups[e] and len(b) < R:
                b.append((int(groups[e].pop()), e))
            if len(b) == R:
                break
        bins.append(b)
    assert len(bins) == N_CORES and all(len(b) == R for b in bins)
    C = max(len({e for _, e in b}) for b in bins)
    perm = np.array([r for b in bins for r, _ in b], dtype=np.int64)
    cand = np.zeros((N_CORES, C), dtype=np.int64)
    masks = np.zeros((N_CORES, R, C), dtype=np.float32)
    for ci, b in enumerate(bins):
        es = sorted({e for _, e in b})
        for j in range(C):
            cand[ci, j] = es[j] if j < len(es) else es[0]
        for r, (_, e) in enumerate(b):
            masks[ci, r, es.index(e)] = 1.0
    return perm, cand, masks, C


def _scan_matrix(d, n1c):
    """A[t, tp] = n1c * (1-d) * d^(tp-t) for tp >= t else 0.  [K, K] f64."""
    A = np.zeros((K, K), dtype=np.float64)
    pw = np.power(d, np.arange(K, dtype=np.float64)) * (1.0 - d) * n1c
    for t in range(K):
        A[t, t:] = pw[: K - t]
    return A


_BUILD_CACHE = {}
_LAST_RESULT = None

NVCH = math.ceil(VC / 512)
WCH = 8            # dma chunks per packed (l, j) weight tile
LMCH = 6           # dma chunks for the packed lm_head tile


def _build_program(C):
    """Build the Bass program. Compile-time param: candidate count C."""
    import concourse.tile as tile
    from concourse import mybir
    from concourse.bacc import Bacc
    from concourse.bass import IndirectOffsetOnAxis
    from concourse.masks import make_identity

    f32 = mybir.dt.float32
    i32 = mybir.dt.int32
    f16 = mybir.dt.float16
    Alu = mybir.AluOpType
    Act = mybir.ActivationFunctionType

    nc = Bacc("TRN2", target_bir_lowering=False, debug=False,
              num_devices=N_CORES)

    emb_t = nc.dram_tensor("emb", [V, H], f16, kind="ExternalInput")
    widx_t = nc.dram_tensor("widx", [P, R], i32, kind="ExternalInput")
    # abm: ab0 for r=0..3 (inv0 and +I folded), then A1 (unscaled), then I
    abm_t = nc.dram_tensor("abm", [P, (R + 2) * P], f16, kind="ExternalInput")
    masks_t = nc.dram_tensor("masks", [R, C], f32, kind="ExternalInput")
    selt_t = nc.dram_tensor("selt", [R, R * P], f16, kind="ExternalInput")
    wts_t = nc.dram_tensor("wts", [L * C * P, HT * H], f16,
                           kind="ExternalInput")
    lmt_t = nc.dram_tensor("lmt", [P, HT * VC], f16, kind="ExternalInput")
    out_t = nc.dram_tensor("logits_part", [B, VC], f32, kind="ExternalOutput")

    with tile.TileContext(nc) as tc:
        with (
            tc.tile_pool(name="const", bufs=1) as cpool,
            tc.tile_pool(name="wp", bufs=1) as wpool,
            tc.tile_pool(name="lmp", bufs=1) as lmpool,
            tc.tile_pool(name="xp", bufs=1) as xpool,
            tc.tile_pool(name="x5p", bufs=1) as x5pool,
            tc.tile_pool(name="small", bufs=1) as spool,
            tc.tile_pool(name="sq", bufs=2) as sqpool,
            tc.tile_pool(name="abp", bufs=2) as abpool,
            tc.tile_pool(name="outp", bufs=2) as opool,
            tc.tile_pool(name="psum", bufs=4, space="PSUM") as ppool,
            tc.tile_pool(name="psum2", bufs=2, space="PSUM") as ppool2,
            tc.tile_pool(name="dram", bufs=1, space="DRAM") as dpool,
        ):
            # ---- small constants on the scalar HWDGE ring: it carries only
            # latency-critical small DMAs, so they never queue behind bulk ----
            widx_sb = cpool.tile([P, R], i32, tag="widx")
            nc.scalar.dma_start(widx_sb[:], widx_t[:])
            abm_sb = cpool.tile([P, (R + 2) * P], f16, tag="abm")
            nc.scalar.dma_start(abm_sb[:], abm_t[:])
            masks_sb = cpool.tile([R, C], f32, tag="masks")
            nc.scalar.dma_start(masks_sb[:], masks_t[:])
            selt_sb = cpool.tile([R, R * P], f16, tag="selt")
            nc.scalar.dma_start(selt_sb[:], selt_t[:])
            amat1 = abm_sb[:, R * P:(R + 1) * P]
            identh = abm_sb[:, (R + 1) * P:(R + 2) * P]

            lm_sb = {}
            # dummy 4-byte collective: pays the one-time cross-core CC
            # rendezvous early, overlapped with the DMA phase, so the real
            # AllGather later starts without it
            dum_in = dpool.tile([1, 1], f32, tag="dumin")
            dum_out = dpool.tile([N_CORES, 1], f32, tag="dumout")
            nc.gpsimd.collective_compute(
                "AllGather", Alu.bypass,
                replica_groups=[list(range(N_CORES))],
                ins=[dum_in.opt()], outs=[dum_out.opt()])

            # ---- gather embeddings ASAP: x0[r] = [128 tokens, H] ----
            x0 = {}
            with nc.named_scope("gather"):
                for r in range(R):
                    xt = xpool.tile([P, H], f16, tag=f"x{r}", name=f"x{r}")
                    nc.gpsimd.indirect_dma_start(
                        out=xt[:], out_offset=None, in_=emb_t[:],
                        in_offset=IndirectOffsetOnAxis(
                            ap=widx_sb[:, r:r + 1], axis=0))
                    x0[r] = xt

            # ---- bulk weights: one INDEPENDENT tile per (l, j, ht) chunk.
            # Chunked writes into a single tile serialize on the tile's
            # completion semaphore, so independent tiles are required for the
            # 16 HW queues to actually run in parallel. All on the sync ring,
            # layer 0 first (earliest consumer).
            w_sb = {}
            for l in range(L):
                for ht in range(HT):
                    for j in range(C):
                        w = wpool.tile([P, H], f16, tag=f"w{l}_{j}_{ht}",
                                       name=f"w{l}_{j}_{ht}")
                        nc.sync.dma_start(
                            w[:], wts_t[(l * C + j) * P:(l * C + j + 1) * P,
                                        ht * H:(ht + 1) * H])
                        w_sb[(l, j, ht)] = w

            # warm both ACT table sets off the critical path
            warm = cpool.tile([1, 2], f32, tag="warm")
            nc.vector.memset(warm[:], 1.0)
            nc.scalar.activation(warm[:, 0:1], warm[:, 0:1], Act.Square)
            nc.scalar.activation(warm[:, 0:1], warm[:, 0:1], Act.Copy)
            nc.scalar.sqrt(warm[:, 1:2], warm[:, 1:2])
            ident = cpool.tile([P, P], f32, tag="ident")
            make_identity(nc, ident[:])

            def pool_ffn(l, xl):
                """xl [R, H] (f16, last-pos states) -> oc [R, H] = selected
                expert FFN output (relu'd), via masked candidates."""
                sq2 = spool.tile([R, H], f32, tag="sq2")
                ss2 = spool.tile([R, 1], f32, tag="ss2")
                nc.scalar.activation(sq2[:], xl[:], Act.Square,
                                     accum_out=ss2[:])
                inv2 = spool.tile([R, 1], f32, tag="inv2")
                nc.vector.tensor_scalar(out=inv2[:], in0=ss2[:],
                                        scalar1=1.0 / H, scalar2=EPS,
                                        op0=Alu.mult, op1=Alu.add)
                nc.vector.reciprocal(out=inv2[:], in_=inv2[:])
                nc.scalar.sqrt(out=inv2[:], in_=inv2[:])
                poolT = {}
                for j in range(C):
                    pm = spool.tile([R, H], f16, tag=f"pm{j}",
                                    name=f"pm{l}_{j}")
                    eng = nc.vector if j % 2 == 0 else nc.gpsimd
                    eng.tensor_scalar(out=pm[:], in0=xl[:],
                                      scalar1=inv2[:],
                                      scalar2=masks_sb[:, j:j + 1],
                                      op0=Alu.mult, op1=Alu.mult)
                    for ht in range(HT):
                        pt_ps = ppool.tile([P, R], f16, tag="psum",
                                           space="PSUM",
                                           name=f"ptps{l}_{j}_{ht}")
                        nc.tensor.transpose(
                            out=pt_ps[:], in_=pm[:, ht * P:(ht + 1) * P],
                            identity=identh[:R, :R])
                        pt = spool.tile([P, R], f16, tag=f"pt{j}_{ht}",
                                        name=f"pt{l}_{j}_{ht}")
                        nc.vector.tensor_copy(out=pt[:], in_=pt_ps[:])
                        poolT[(j, ht)] = pt
                oc = spool.tile([R, H], f16, tag="oc")
                pe = [ppool.tile([R, 512], f32, tag="psum", space="PSUM",
                                 name=f"pe{l}_{d}") for d in range(DC)]
                n = 0
                for j in range(C):
                    for ht in range(HT):
                        for d in range(DC):
                            nc.tensor.matmul(
                                pe[d][:], lhsT=poolT[(j, ht)][:],
                                rhs=w_sb[(l, j, ht)][:, d * 512:(d + 1) * 512],
                                start=(n == 0), stop=(n == C * HT - 1))
                        n += 1
                for d in range(DC):
                    nc.vector.tensor_scalar(
                        out=oc[:, d * 512:(d + 1) * 512], in0=pe[d][:],
                        scalar1=0.0, scalar2=None, op0=Alu.max)
                return oc

            # ================= layer 0 =================
            with nc.named_scope("layer0"):
                xl0 = spool.tile([R, H], f16, tag="xl0")
                x05 = {}
                for r in range(R):
                    ps = ppool2.tile([P, H], f32, tag="psum2", space="PSUM",
                                     name=f"ps0_{r}")
                    for d in range(DC):
                        nc.tensor.matmul(
                            ps[:, d * 512:(d + 1) * 512],
                            lhsT=abm_sb[:, r * P:(r + 1) * P],
                            rhs=x0[r][:, d * 512:(d + 1) * 512],
                            start=True, stop=True)
                    xt = x5pool.tile([P, H], f16, tag=f"x5{r}", name=f"x5{r}")
                    nc.scalar.activation(xt[:], ps[:], Act.Copy)
                    nc.gpsimd.dma_start(out=xl0[r:r + 1, :],
                                        in_=xt[P - 1:P, :])
                    x05[r] = xt
                oc0 = pool_ffn(0, xl0)
                # lm_head tiles dispatch only now (scalar ring): the expert
                # weights gating FFN0/FFN1 get the full HBM bandwidth first,
                # and these still land well before the head needs them
                for ht in range(HT):
                    lm = lmpool.tile([P, VC], f16, tag=f"lm{ht}",
                                     name=f"lm{ht}")
                    nc.scalar.dma_start(lm[:], lmt_t[:, ht * VC:(ht + 1) * VC])
                    lm_sb[ht] = lm
                # residual broadcast of oc0 into every window position
                for r in range(R):
                    for d in range(DC):
                        ob = ppool.tile([P, 512], f32, tag="psum",
                                        space="PSUM", name=f"ob{r}_{d}")
                        nc.tensor.matmul(
                            ob[:], lhsT=selt_sb[:, r * P:(r + 1) * P],
                            rhs=oc0[:, d * 512:(d + 1) * 512],
                            start=True, stop=True)
                        nc.vector.tensor_tensor(
                            out=x05[r][:, d * 512:(d + 1) * 512],
                            in0=x05[r][:, d * 512:(d + 1) * 512],
                            in1=ob[:], op=Alu.add)

            # ================= layer 1 =================
            with nc.named_scope("layer1"):
                xl1 = spool.tile([R, H], f16, tag="xl1")
                for r in range(R):
                    ssum = spool.tile([P, 1], f32, tag=f"ssum{r}")
                    sq = sqpool.tile([P, H], f32, tag="sq")
                    nc.scalar.activation(sq[:], x05[r][:], Act.Square,
                                         accum_out=ssum[:])
                    inv1 = spool.tile([P, 1], f32, tag=f"inv1{r}")
                    nc.vector.tensor_scalar(out=inv1[:], in0=ssum[:],
                                            scalar1=1.0 / H, scalar2=EPS,
                                            op0=Alu.mult, op1=Alu.add)
                    nc.vector.reciprocal(out=inv1[:], in_=inv1[:])
                    nc.scalar.sqrt(out=inv1[:], in_=inv1[:])
                    # only the scan's LAST row is needed downstream, so
                    # contract with just column 127 of the scan matrix:
                    # xl1[r] = sum_t (A1[t,127]*inv1[t] + I[t,127]) * x1[t]
                    abL = abpool.tile([P, 1], f16, tag="abL", name=f"abL{r}")
                    nc.vector.tensor_scalar(out=abL[:],
                                            in0=amat1[:, P - 1:P],
                                            scalar1=inv1[:], scalar2=None,
                                            op0=Alu.mult)
                    nc.vector.tensor_tensor(out=abL[:], in0=abL[:],
                                            in1=identh[:, P - 1:P],
                                            op=Alu.add)
                    xt1 = spool.tile([1, H], f16, tag=f"xt1_{r}",
                                     name=f"xt1_{r}")
                    for d in range(DC):
                        psl = ppool.tile([1, 512], f32, tag="psum",
                                         space="PSUM", name=f"psl{r}_{d}")
                        nc.tensor.matmul(
                            psl[:], lhsT=abL[:],
                            rhs=x05[r][:, d * 512:(d + 1) * 512],
                            start=True, stop=True)
                        if r < 2:
                            nc.scalar.activation(
                                xt1[:, d * 512:(d + 1) * 512], psl[:],
                                Act.Copy)
                        else:
                            nc.vector.tensor_copy(
                                out=xt1[:, d * 512:(d + 1) * 512], in_=psl[:])
                    nc.gpsimd.dma_start(out=xl1[r:r + 1, :], in_=xt1[:])
                oc1 = pool_ffn(1, xl1)

            with nc.named_scope("fin"):
                fin = spool.tile([R, H], f32, tag="fin")
                nc.vector.tensor_tensor(out=fin[:], in0=xl1[:], in1=oc1[:],
                                        op=Alu.add)
                sq3 = spool.tile([R, H], f32, tag="sq3")
                ss3 = spool.tile([R, 1], f32, tag="ss3")
                nc.scalar.activation(sq3[:], fin[:], Act.Square,
                                     accum_out=ss3[:])
                inv3 = spool.tile([R, 1], f32, tag="inv3")
                nc.vector.tensor_scalar(out=inv3[:], in0=ss3[:],
                                        scalar1=1.0 / H, scalar2=EPS,
                                        op0=Alu.mult, op1=Alu.add)
                nc.vector.reciprocal(out=inv3[:], in_=inv3[:])
                nc.scalar.sqrt(out=inv3[:], in_=inv3[:])
                finn = spool.tile([R, H], f16, tag="finn")
                nc.vector.tensor_scalar(out=finn[:], in0=fin[:],
                                        scalar1=inv3[:], scalar2=None,
                                        op0=Alu.mult)

            with nc.named_scope("ag"):
                ag_in = dpool.tile([R, H], f16, tag="agin")
                ag_out = dpool.tile([B, H], f16, tag="agout")
                nc.gpsimd.dma_start(ag_in[:], finn[:])
                nc.gpsimd.collective_compute(
                    "AllGather", Alu.bypass,
                    replica_groups=[list(range(N_CORES))],
                    ins=[ag_in.opt()], outs=[ag_out.opt()])
                fin_all = spool.tile([B, H], f16, tag="finall")
                nc.gpsimd.dma_start(fin_all[:], ag_out[:])

            with nc.named_scope("head"):
                fT = {}
                for ht in range(HT):
                    ft_ps = ppool.tile([P, B], f16, tag="psum", space="PSUM",
                                       name=f"ftps{ht}")
                    nc.tensor.transpose(out=ft_ps[:],
                                        in_=fin_all[:, ht * P:(ht + 1) * P],
                                        identity=identh[:B, :B])
                    ft = spool.tile([P, B], f16, tag=f"ft{ht}", name=f"ft{ht}")
                    nc.vector.tensor_copy(out=ft[:], in_=ft_ps[:])
                    fT[ht] = ft
                for vch in range(NVCH):
                    v0 = vch * 512
                    nv = min(512, VC - v0)
                    pv = ppool.tile([B, nv], f32, tag="psum", space="PSUM",
                                    name=f"pv{vch}")
                    for ht in range(HT):
                        nc.tensor.matmul(pv[:], lhsT=fT[ht][:],
                                         rhs=lm_sb[ht][:, v0:v0 + nv],
                                         start=(ht == 0), stop=(ht == HT - 1))
                    ov = opool.tile([B, nv], f32, tag="ov")
                    nc.vector.tensor_copy(out=ov[:], in_=pv[:])
                    nc.gpsimd.dma_start(out_t[:, v0:v0 + nv], ov[:])

    if not nc.is_finalized():
        nc.finalize()
    return nc


def _get_program(C):
    if C not in _BUILD_CACHE:
        _BUILD_CACHE[C] = _build_program(C)
    return _BUILD_CACHE[C]


def _prepare(windows, hemis, experts, emb, norm1_w, decay_logit, norm2_w,
             Wexp, final_norm_w, lm_head):
    """Host-side prep: returns (nc, in_maps, perm)."""
    del hemis
    windows = np.asarray(windows)
    experts = np.asarray(experts)
    emb = np.asarray(emb, dtype=np.float32)
    Wexp = np.asarray(Wexp, dtype=np.float32)
    lm_head = np.asarray(lm_head, dtype=np.float32)

    d = _sigmoid64(decay_logit)  # [L, H]
    assert np.all(np.abs(d - d.mean(axis=1, keepdims=True)) < 1e-12), \
        "kernel assumes channel-uniform decay"
    dly = d.mean(axis=1)
    assert K * float(dly.max()) ** K < 1e-4, "K window too small for decay"
    n1c = [_uniform_const(np.asarray(norm1_w)[l]) for l in range(L)]
    n2c = [_uniform_const(np.asarray(norm2_w)[l]) for l in range(L)]
    fnc = _uniform_const(final_norm_w)
    assert all(c is not None for c in n1c + n2c) and fnc is not None, \
        "kernel assumes constant norm weight vectors"
    assert n2c[0] == n2c[1], "per-layer norm2 consts differ; masks are shared"

    A0 = _scan_matrix(float(dly[0]), n1c[0])
    A1 = _scan_matrix(float(dly[1]), n1c[1])
    perm, cand, masks, C = _pack_rows(experts)

    nc = _get_program(C)

    emb_m = np.ascontiguousarray(emb.astype(np.float16))
    # inverse rms of the (dtype-rounded) embedding rows for layer 0
    embf = emb_m.astype(np.float32)
    norms = (embf * embf).mean(axis=1) + np.float32(EPS)
    inv_emb = (1.0 / np.sqrt(norms)).astype(np.float64)  # [V]

    # lm_head packed: lmt[p, ht*VC + v] = (lm_head.T * fnc)[ht*128+p, v_core]
    lmt_full = (lm_head.T * np.float32(fnc)).astype(np.float16)  # [H, V]

    selt = np.zeros((R, R * P), dtype=np.float16)
    for r in range(R):
        selt[r, r * P:(r + 1) * P] = 1.0
    ident = np.eye(P, dtype=np.float16)

    in_maps = []
    for ci in range(N_CORES):
        rows = perm[ci * R:(ci + 1) * R]
        win = windows[rows][:, S - K:]  # [R, K]
        widx = np.ascontiguousarray(win.T).astype(np.int32)  # [P, R]
        abm = np.empty((P, (R + 2) * P), dtype=np.float16)
        for r in range(R):
            ab0 = A0 * inv_emb[win[r]][:, None] + np.eye(K)
            abm[:, r * P:(r + 1) * P] = ab0.astype(np.float16)
        abm[:, R * P:(R + 1) * P] = A1.astype(np.float16)
        abm[:, (R + 1) * P:(R + 2) * P] = ident
        wts = np.empty((L * C * P, HT * H), dtype=np.float16)
        for l in range(L):
            for j in range(C):
                wt = Wexp[l, cand[ci, j]].T.astype(np.float16)  # [H, H]
                wts[(l * C + j) * P:(l * C + j + 1) * P, :] = (
                    wt.reshape(HT, P, H).transpose(1, 0, 2).reshape(P, HT * H))
        lmc = lmt_full[:, ci * VC:(ci + 1) * VC]  # [H, VC]
        lmt = np.ascontiguousarray(
            lmc.reshape(HT, P, VC).transpose(1, 0, 2).reshape(P, HT * VC))
        in_maps.append(dict(
            emb=emb_m,
            widx=widx,
            abm=np.ascontiguousarray(abm),
            masks=np.ascontiguousarray(masks[ci] * np.float32(n2c[0])),
            selt=selt,
            wts=np.ascontiguousarray(wts),
            lmt=lmt,
        ))
    return nc, in_maps, perm


def _assemble(results, perm):
    logits_sorted = np.concatenate(
        [results[ci]["logits_part"] for ci in range(N_CORES)], axis=1)
    logits = np.empty((B, V), dtype=np.float32)
    logits[perm] = logits_sorted
    return logits


def kernel(**inputs):
    from concourse.bass_utils import run_bass_kernel_spmd

    nc, in_maps, perm = _prepare(**inputs)
    res = run_bass_kernel_spmd(nc, in_maps, core_ids=list(range(N_CORES)))
    global _LAST_RESULT
    _LAST_RESULT = res
    return _assemble(res.results, perm)
